# revision 12
# baseline (speedup 1.0000x reference)
"""Trainium2 Bass kernel for nn_ChannelAttention.

Reference computation (B=2, W=D=H=32, C=256, N=W*D*H=32768):
  4 branches i in {Q,K,J,V}:  Y_i = relu(BN_i(x @ W_i + b_i))  (1x1x1 conv + BN folded)
  raw reshape (B,W,D,H,C) -> (B,C,N):  with s = 128*r + j (j in [0,128)):
     Resh[r, (j,c)] = Y[s=128r+j, c]
  m1 = K @ Q^T, m2 = K @ J^T   (contraction over (j, c))
  aff = sigmoid(m1 @ m2);  out = gamma * (aff @ V).reshape + x

Sharding: 8 cores = 2 batches x 4 quarters of the within-block offset j
(core g: batch g//4, j in [32*(g%4), 32*(g%4)+32)).  Each core computes
partial Gram matrices over its local (t, c) contraction slice; a 4-core
bf16 AllReduce per batch completes m1/m2; the V branch runs in the
collective's shadow; affinity apply then only needs the local shard.

Precision: the gamma=1e-4 residual scale damps the whole attention path
~1e4x below the output magnitude, so the heavy matmuls run in fp8
(DoubleRow perf mode: contraction 256 in one PE pass) with fp32 PSUM
accumulation; the Grams travel bf16 through the AllReduce; affT carries
gamma in fp8e5 (1e-4 is subnormal in e4m3 but normal in e5m2); the
residual add and output run in bf16.

Per-core layouts (local t in [0,32), block r in [0,256), c in [0,256)):
  xs    DRAM [r, t, c] bf16       -- residual source
  xst8  DRAM [cc, p, t, r] fp8e4  -- x^T: value x[r, t, 128cc+p]
  xt8   SBUF [128, 2, T, R]       -- DoubleRow ifmap (cc = contraction pair)
  qkj8  SBUF [128, 2, T, 3, R] fp8-- branch outputs [c_lo, c_hi, t, (q,j,k), r]
  mq    PSUM [128, 2, 512] f32    -- [r1_lo, r1_hi, (m1 | m2)] Gram accumulators
        gram MM: lhsT = k-slice (shared stationary), rhs = q|j concat
  msb   [128, (m1T chunks | m2 chunks), r] bf16 -> 256 KB AllReduce
  v8    SBUF [128, 2, T, C] fp8e4 -- V natural [r_lo, r_hi, t, c]
  afft8 [128, 2, R] fp8e5         -- gamma * sigmoid(aux)^T
  apply: one DoubleRow MM per (rc, tp): psw[r, (t,c)] = sum_r' affT[r',r] V[r',(t,c)]
"""

import numpy as np
import ml_dtypes

import concourse.bass as bass
import concourse.bacc as bacc
import concourse.mybir as mybir
import concourse.tile as tile
from concourse.bass_utils import run_bass_kernel_spmd

BN_EPS = 1e-3
BF16 = mybir.dt.bfloat16
F32 = mybir.dt.float32
FP8 = mybir.dt.float8e4
FP8E5 = mybir.dt.float8e5
AF = mybir.ActivationFunctionType
ALU = mybir.AluOpType
DR = mybir.MatmulPerfMode.DoubleRow

C = 256          # channels
R = 256          # blocks (rows of the raw-reshaped matrix)
T = 32           # within-block offsets per core (128 / 4 cores per batch)
NCORES = 8

LAST_RESULT = None  # BassKernelResults of the most recent run (for profiling)


def _build_program(gamma: float):
    nc = bacc.Bacc("TRN2", target_bir_lowering=False, debug=False,
                   num_devices=NCORES)

    xs = nc.dram_tensor("xs", [R, T, C], BF16, kind="ExternalInput")
    xst8 = nc.dram_tensor("xst8", [2, 128, T, R], FP8, kind="ExternalInput")
    wall8 = nc.dram_tensor("wall8", [128, 2, 4, C], FP8, kind="ExternalInput")
    bqkj = nc.dram_tensor("bqkj", [128, 3, 2], F32, kind="ExternalInput")
    bv = nc.dram_tensor("bv", [1, 2 * C], BF16, kind="ExternalInput")
    identt = nc.dram_tensor("identt", [128, 128], BF16, kind="ExternalInput")
    xout = nc.dram_tensor("xout", [R, T, C], BF16, kind="ExternalOutput")

    sv = gamma * 2.0 ** 13  # gamma split: afft8 carries 2^-13, V carries sv

    with tile.TileContext(nc) as tc:
        with (
            tc.tile_pool(name="const", bufs=1) as const,
            tc.tile_pool(name="big", bufs=1) as big,
            tc.tile_pool(name="io", bufs=4) as io,
            tc.tile_pool(name="xres", bufs=8) as xres,
            tc.tile_pool(name="outp", bufs=3) as outp,
            tc.tile_pool(name="workps", bufs=4, space="PSUM") as workps,
            tc.tile_pool(name="mps", bufs=1, space="PSUM") as mps,
            tc.tile_pool(name="tps", bufs=2, space="PSUM") as tps,
            tc.tile_pool(name="dram", bufs=1, space="DRAM") as dram,
        ):
            # ---- X^T first (small leading chunks so matmuls start early)
            xt8 = big.tile([128, 2, T, R], FP8, tag="xt8", name="xt8")
            w_sb = const.tile([128, 2, 4, C], FP8)
            b_sb = const.tile([128, 3, 2], F32)
            bv_sb = const.tile([1, 2 * C], BF16)
            id_sb = const.tile([128, 128], BF16)
            nc.scalar.dma_start(out=w_sb, in_=wall8[:, :, :, :])
            nc.scalar.dma_start(out=b_sb, in_=bqkj[:, :, :])
            nc.scalar.dma_start(out=bv_sb, in_=bv[:, :])
            nc.scalar.dma_start(out=id_sb, in_=identt[:, :])
            for t0, tn in [(0, 2), (2, 2), (4, 4), (8, 8), (16, 8), (24, 8)]:
                for cc in range(2):
                    nc.sync.dma_start(
                        out=xt8[:, cc, t0:t0 + tn, :],
                        in_=xst8[cc, :, t0:t0 + tn, :],
                    )
            ones = const.tile([1, 128], BF16)
            nc.vector.memset(ones, 1.0)

            qkj8 = big.tile([128, 2, T, 3, R], FP8, tag="qkj8", name="qkj8")
            v8 = big.tile([128, 2, T, C], FP8, tag="v8", name="v8")

            # ---- Gram accumulators: [r1-half ch, (m1 | m2)] per bank
            mq = mps.tile([128, 2, 512], F32, tag="mq")

            # ---- phase 1: Q/J/K branches (fp8 DoubleRow), Gram ----
            for tp in range(16):
                for br in range(3):
                    for co in range(2):
                        ps = workps.tile([128, 512], F32, tag="work")
                        nc.tensor.matmul(
                            ps,
                            w_sb[:, :, br, 128 * co:128 * (co + 1)],
                            xt8[:, :, 2 * tp:2 * (tp + 1), :],
                            start=True, stop=True, perf_mode=DR,
                        )
                        dest = qkj8[:, co, 2 * tp:2 * (tp + 1), br, :]
                        if br == 0 or (br == 1 and co == 0):
                            nc.scalar.activation(dest, ps, AF.Relu,
                                                 bias=b_sb[:, br, co:co + 1])
                        else:
                            nc.vector.tensor_scalar(
                                dest, ps, b_sb[:, br, co:co + 1], 0.0,
                                ALU.add, ALU.max)

                for ti in range(2):
                    t = 2 * tp + ti
                    for ch in range(2):
                        first = (tp == 0 and ti == 0)
                        last = (tp == 15 and ti == 1)
                        # mq[:, ch, 0:256] += k^T q ; mq[:, ch, 256:512] += k^T j
                        nc.tensor.matmul(
                            mq[:, ch, :],
                            qkj8[:, :, t, 2, 128 * ch:128 * (ch + 1)],
                            qkj8[:, :, t, 0:2, :],
                            start=first, stop=last, perf_mode=DR,
                        )

            # ---- phase 2: evict Grams, transpose m1, bf16 AllReduce ----
            msb = const.tile([128, 4, R], BF16)
            tmp1 = const.tile([128, 2, R], BF16)
            nc.scalar.activation(tmp1, mq[:, :, 0:256], AF.Copy)
            nc.scalar.activation(msb[:, 2:4, :], mq[:, :, 256:512], AF.Copy)
            for hh in range(2):
                for kk in range(2):
                    pst = tps.tile([128, 128], BF16, tag="tps")
                    nc.tensor.transpose(
                        pst, tmp1[:, hh, 128 * kk:128 * (kk + 1)], id_sb)
                    nc.scalar.activation(msb[:, kk, 128 * hh:128 * (hh + 1)],
                                         pst, AF.Copy)
            cc_in = dram.tile([128, 4, R], BF16)
            cc_out = dram.tile([128, 4, R], BF16)
            nc.sync.dma_start(out=cc_in, in_=msb)
            nc.gpsimd.collective_compute(
                "AllReduce",
                ALU.add,
                replica_groups=[[0, 1, 2, 3], [4, 5, 6, 7]],
                ins=[cc_in.opt()],
                outs=[cc_out.opt()],
            )
            m_red = const.tile([128, 4, R], BF16)
            nc.sync.dma_start(out=m_red, in_=cc_out)

            # prefetch the residual chunks during the AllReduce window
            xres_pre = []
            for rc in range(2):
                for g in range(4):
                    xresc = xres.tile([128, 8, C], BF16, tag="xresc",
                                      name="xresc")
                    nc.scalar.dma_start(
                        out=xresc,
                        in_=xs[128 * rc:128 * (rc + 1), 8 * g:8 * (g + 1), :])
                    xres_pre.append(xresc)

            # ---- phase 2b: V branch, fp8 DoubleRow (overlaps AllReduce) ----
            for tp in range(16):
                for rh in range(2):
                    psv = workps.tile([128, 2, C], F32, tag="work")
                    for ti in range(2):
                        t = 2 * tp + ti
                        nc.tensor.matmul(
                            psv[:, ti, :],
                            xt8[:, :, t, 128 * rh:128 * (rh + 1)],
                            w_sb[:, :, 3, :],
                            start=(ti == 0), stop=False, perf_mode=DR)
                    nc.tensor.matmul(psv, ones, bv_sb,
                                     start=False, stop=True)
                    dest = v8[:, rh, 2 * tp:2 * (tp + 1), :]
                    if rh == 0:
                        nc.scalar.activation(dest, psv, AF.Relu, scale=sv)
                    else:
                        nc.vector.tensor_scalar(dest, psv, sv, 0.0,
                                                ALU.mult, ALU.max)

            # warm the sigmoid activation table during the AllReduce wait
            warm = io.tile([1, 1], F32, tag="warm")
            nc.scalar.activation(warm, ones[0:1, 0:1], AF.Sigmoid)

            # ---- phase 3: auxT chunks = m2-chunk^T @ m1T; affT = gamma*sigmoid
            afft8 = const.tile([128, 2, R], FP8E5)
            psa = workps.tile([128, 2, R], F32, tag="work")
            for pch in range(2):
                for kch in range(2):
                    nc.tensor.matmul(
                        psa[:, pch, :],
                        m_red[:, 2 + kch, 128 * pch:128 * (pch + 1)],
                        m_red[:, kch, :],
                        start=(kch == 0), stop=(kch == 1))
            aff_f = io.tile([128, 2, R], F32, tag="afff")
            nc.scalar.activation(aff_f, psa, AF.Sigmoid)
            nc.vector.tensor_scalar_mul(afft8, aff_f, 2.0 ** -13)

            # ---- phase 4: apply (fp8 DoubleRow) + bf16 residual ----
            for rc in range(2):
                for g in range(4):
                    xresc = xres_pre[4 * rc + g]
                    outc = outp.tile([128, 8, C], BF16, tag="outc")
                    for tq in range(4):
                        tp = 4 * g + tq
                        psw = workps.tile([128, 512], F32, tag="work")
                        scalar_path = (tq % 2 == 1)
                        nc.tensor.matmul(
                            psw,
                            afft8[:, :, 128 * rc:128 * (rc + 1)],
                            v8[:, :, 2 * tp:2 * (tp + 1), :],
                            start=True, stop=not scalar_path, perf_mode=DR)
                        if scalar_path:
                            # accumulate the residual on the PE, evict on ACT
                            nc.tensor.matmul(
                                psw, id_sb, xresc[:, 2 * tq:2 * (tq + 1), :],
                                start=False, stop=True)
                            nc.scalar.activation(
                                outc[:, 2 * tq:2 * (tq + 1), :], psw, AF.Copy)
                        else:
                            nc.vector.tensor_tensor(
                                outc[:, 2 * tq:2 * (tq + 1), :], psw,
                                xresc[:, 2 * tq:2 * (tq + 1), :], ALU.add)
                    nc.sync.dma_start(
                        out=xout[128 * rc:128 * (rc + 1), 8 * g:8 * (g + 1), :],
                        in_=outc)

    nc.compile()
    return nc


def _prep_host(conv_w, conv_b, bn_scale, bn_offset, bn_mean, bn_var):
    """Fold BN into the conv weights (float64 then cast).

    Device branch order is (q, j, k, v) = reference (query, judge, key,
    value) so that q|j sit adjacent as the Gram moving operand.
    """
    w = conv_w.astype(np.float64)
    b = conv_b.astype(np.float64)
    s = bn_scale.astype(np.float64)
    o = bn_offset.astype(np.float64)
    m = bn_mean.astype(np.float64)
    v = bn_var.astype(np.float64)
    r = s / np.sqrt(v + BN_EPS)                      # (4, C)
    wp = w * r[:, None, :]                           # (4, C, C), scales cout
    bp = (b - m) * r + o                             # (4, C)
    perm = [0, 2, 1, 3]                              # q, j, k, v
    wall8 = np.ascontiguousarray(
        wp[perm].reshape(4, 2, 128, C).transpose(2, 1, 0, 3)
    ).astype(ml_dtypes.float8_e4m3)                  # [p, cc, br, f]
    bqkj_host = np.ascontiguousarray(
        bp[perm][:3].reshape(3, 2, 128).transpose(2, 0, 1)
    ).astype(np.float32)                             # [p, br, co]
    bv_host = np.tile(bp[3:4], (1, 2)).astype(ml_dtypes.bfloat16)  # (1, 2C)
    id_host = np.eye(128, dtype=ml_dtypes.bfloat16)
    return wall8, bqkj_host, bv_host, id_host


def _shard_inputs(x):
    """Per-core shards: core g -> batch g//4, quarter q = g%4 of offset j."""
    B = x.shape[0]
    xr = x.reshape(B, R, 4, T, C)           # [b, r, q, t, c]
    shards = []
    for g in range(NCORES):
        b, q = g // 4, g % 4
        shard = np.ascontiguousarray(xr[b, :, q]).astype(np.float32)
        xst8 = np.ascontiguousarray(
            shard.transpose(2, 1, 0).reshape(2, 128, T, R)
        ).astype(ml_dtypes.float8_e4m3)     # [cc, p, t, r]
        shards.append(dict(
            xs=shard.astype(ml_dtypes.bfloat16),
            xst8=xst8,
        ))
    return shards


def kernel(x, conv_w, conv_b, bn_scale, bn_offset, bn_mean, bn_var, gamma,
           **_unused):
    x = np.asarray(x)
    B, W, D, H, Cc = x.shape
    assert (B, W, D, H, Cc) == (2, 32, 32, 32, 256), x.shape
    gamma_f = float(np.asarray(gamma))

    wall8, bqkj_host, bv_host, id_host = _prep_host(
        np.asarray(conv_w), np.asarray(conv_b), np.asarray(bn_scale),
        np.asarray(bn_offset), np.asarray(bn_mean), np.asarray(bn_var))

    nc = _build_program(gamma_f)

    in_maps = []
    for shard in _shard_inputs(x):
        in_maps.append(dict(
            wall8=wall8, bqkj=bqkj_host, bv=bv_host, identt=id_host,
            **shard,
        ))

    res = run_bass_kernel_spmd(nc, in_maps, core_ids=list(range(NCORES)))
    global LAST_RESULT
    LAST_RESULT = res

    out = np.empty((B, R, 4, T, Cc), dtype=np.float32)
    for g in range(NCORES):
        b, q = g // 4, g % 4
        out[b, :, q] = np.asarray(res.results[g]["xout"]).astype(np.float32)
    return out.reshape(B, W, D, H, Cc)
